# revision 40
# baseline (speedup 1.0000x reference)
"""Trainium2 Bass kernel for nn_CorefModel (GNN message passing, 8 NeuronCores).

Sharding: constituents 2048/core, tokens 1024/core (row shards). Per-iteration
node-feature tables (hf, hb, hk) are produced by per-core GEMMs, AllGathered
into a full per-core HBM copy, and edge gathers (dma_gather) read full rows
from the local copy. GAT edge aggregation runs as one-hot scatter matmuls on
the TensorEngine over destination-sorted edge chunks; the softmax denominator
is accumulated by the same one-hot against exp values.

Perf design (v8):
- Feature tables tf/tb/tk stored FP8 (e4m3) with bf16 sd columns appended:
  row = [vals fp8 | sd bf16 | pad]. Halves AllGather wire bytes (the serial
  collective spine) and edge-gather HBM traffic; logits stay bf16.
- All value layouts NATURAL head order. The per-edge alpha is folded into
  per-head one-hot planes (oha = oh * exp, DVE bf16 2x) and the PE consumes
  the raw gathered fp8 rows as the matmul moving operand (mixed-dtype
  bf16-lhsT x fp8-rhs matmul, validated on HW).
- PV accumulation uses ONE open PSUM accumulation group per 2KB bank
  (head-pairs, [128,2,512] f32 tiles). Two concurrently-open groups in one
  bank corrupt each other (hard-won lesson; see head-parity bug).
- Edge-side one-hot generated on-chip via DVE is_equal against an iota
  const; dst-side (ohd) streamed from a host-precomputed plane.
- hf/hb production GEMMs run fp8 DoubleRow over k-tile pairs (2 k-tiles
  per matmul), k-outer/n-inner over a 4-bank PSUM tile.
- Logit strip: one-hot matmul (sd_d scatter) + DVE add of gathered sd_s.
- P7 blend consumes tok_cons per 128-row window (no full-table barrier).
PSUM accumulation fp32 throughout; final blend in fp32.
"""
import sys, os
sys.path.insert(0, '/opt/trn_rl_repo')
import math
import numpy as np
import ml_dtypes

import concourse.bass as bass
from concourse import bacc
import concourse.tile as tile
from concourse import mybir
from concourse.bass_utils import run_bass_kernel_spmd

BF = ml_dtypes.bfloat16
F8 = ml_dtypes.float8_e4m3fn
F32 = np.float32

H = 8
NT, NC = 8192, 16384
DH, DCH = 768, 128
D = 2 * DH + DCH          # 1664
NCORES = 8
NCL, NTL = NC // NCORES, NT // NCORES     # 2048, 1024
WCC, WCT = NCL // 128, NTL // 128         # 16, 8
SWF = 1792                 # cc table row stride in BYTES (fp8 elems)
SWK = 1024                 # hk table row stride in BYTES
SDF = 832                  # bf16-col offset of sd_s in cc rows (byte 1664)
SDK = 384                  # bf16-col offset of sd_k in ct rows (byte 768)

f32 = mybir.dt.float32
bf16 = mybir.dt.bfloat16
fp8 = mybir.dt.float8e4
i16 = mybir.dt.int16
AF = mybir.ActivationFunctionType
OP = mybir.AluOpType


# ---------------------------------------------------------------- host prep --

def _bdiag(a):
    h, dh = a.shape
    m = np.zeros((h * dh, h), F32)
    for i in range(h):
        m[i * dh:(i + 1) * dh, i] = a[i]
    return m


def _permC(W, h, dh):
    # natural col f = hh*dh+d  ->  head-minor col p = d*h+hh
    r = W.shape[0]
    return np.ascontiguousarray(W.reshape(r, h, dh).transpose(0, 2, 1).reshape(r, h * dh))


def _permR(W, h, dh):
    c = W.shape[1]
    return np.ascontiguousarray(W.reshape(h, dh, c).transpose(1, 0, 2).reshape(h * dh, c))


def _permV(v, h, dh):
    return np.ascontiguousarray(v.reshape(h, dh).T.reshape(h * dh))


def _idx_plane(idxs):
    """[128, n/16] int16: idx i at (i%16, i//16), replicated across 8 Q7 groups."""
    n = len(idxs)
    plane = np.zeros((128, n // 16), np.int16)
    plane[np.arange(n) % 16, np.arange(n) // 16] = idxs
    for g in range(1, 8):
        plane[16 * g:16 * (g + 1)] = plane[:16]
    return plane


def _prep_edges(srcs, dsts, wins):
    """Per edge set: sort by dst, split into per-(core, 128-dst-window) groups.
    Returns uniform chunks/window C and per-core index + dst-local planes:
      idx [128, blocks*E/16] int16 : source row ids (gather plane)
      dl  [128, blocks*C]   bf16  : dl[e%128, blk*C + e//128] = dst%128 or -1
    """
    nsets = len(srcs)
    percw = [[[None] * wins for _ in range(nsets)] for _ in range(NCORES)]
    maxe = 0
    for s in range(nsets):
        src, dst = srcs[s], dsts[s]
        gwin = dst // 128
        order = np.argsort(gwin, kind='stable')
        src_s, dst_s, gwin_s = src[order], dst[order], gwin[order]
        bounds = np.searchsorted(gwin_s, np.arange(NCORES * wins + 1))
        for c in range(NCORES):
            for w in range(wins):
                g = c * wins + w
                lo, hi = bounds[g], bounds[g + 1]
                maxe = max(maxe, hi - lo)
                percw[c][s][w] = (src_s[lo:hi], dst_s[lo:hi] % 128)
    C = max(1, math.ceil(maxe / 128))
    E = C * 128
    ar128 = np.arange(128)
    out = []
    for c in range(NCORES):
        nblk = nsets * wins
        idxp = np.zeros((128, nblk * (E // 16)), np.int16)
        dl = np.full((128, nblk * C), -1.0, BF)
        ohd = np.zeros((128, nblk * E), BF)
        for s in range(nsets):
            for w in range(wins):
                src_e, dloc = percw[c][s][w]
                k = len(src_e)
                blk = s * wins + w
                sp = np.zeros(E, np.int16); sp[:k] = src_e.astype(np.int16)
                dp = np.full(E, -1.0, np.float32); dp[:k] = dloc
                idxp[:, blk * (E // 16):(blk + 1) * (E // 16)] = _idx_plane(sp)
                dl[:, blk * C:(blk + 1) * C] = dp.reshape(C, 128).T.astype(BF)
                eq = (dp.reshape(C, 128)[:, :, None] == ar128[None, None, :])  # [c,e,j]
                ohd[:, blk * E:(blk + 1) * E] = \
                    eq.transpose(2, 0, 1).reshape(128, E).astype(BF)
        out.append(dict(idx=idxp, dl=dl, ohd=ohd))
    return C, out


def _host_prep(inp):
    tok = np.asarray(inp['token_embeddings'], F32)
    starts = np.asarray(inp['constituent_starts']).astype(np.int64)
    ends = np.asarray(inp['constituent_ends']).astype(np.int64)
    labels = np.asarray(inp['constituent_labels']).astype(np.int64)
    label_emb = np.asarray(inp['cons_type_table'], F32)[labels]        # [NC, 128]

    Wf = np.asarray(inp['Wf'], F32); Wb = np.asarray(inp['Wb'], F32)
    Wk = np.asarray(inp['Wk'], F32); Wq = np.asarray(inp['Wq'], F32)
    # tables: NATURAL column order (head-major) everywhere; sd cols appended.
    Wf_ext = np.concatenate([Wf,
                             Wf @ _bdiag(np.asarray(inp['af_s'], F32)),
                             Wf @ _bdiag(np.asarray(inp['af_d'], F32))], 1).astype(F8)
    Wb_ext = np.concatenate([Wb,
                             Wb @ _bdiag(np.asarray(inp['ab_s'], F32)),
                             Wb @ _bdiag(np.asarray(inp['ab_d'], F32))], 1).astype(F8)
    Wk_ext = np.concatenate([Wk,
                             Wk @ _bdiag(np.asarray(inp['act_s'], F32)),
                             np.zeros((D, 8), F32)], 1).astype(F8)     # [1664, 784]
    Wq_ad = (Wq @ _bdiag(np.asarray(inp['act_d'], F32))).astype(BF)
    w1 = np.asarray(inp['attn_w1'], F32).astype(F8)                    # [1664, 1024]
    w2c = np.asarray(inp['attn_w2'], F32).reshape(8, 128).T.copy()
    w2r = np.concatenate([np.repeat(w2c, 128, axis=1),
                          np.repeat(-w2c, 128, axis=1)], 1).astype(BF)  # [128, 2*8*128]
    b1c = np.asarray(inp['attn_b1'], F32).reshape(8, 128).T.copy()     # [128, 8] f32
    fuse_w = np.asarray(inp['fuse_w'], F32).astype(BF)                 # [1536, 768]
    fb_row = np.asarray(inp['fuse_b'], F32).reshape(1, DH).astype(BF)

    cons = np.concatenate([tok[starts], tok[ends], label_emb], 1)      # [NC, 1664] f32

    cc_src = np.asarray(inp['cc_src']); cc_dst = np.asarray(inp['cc_dst'])
    ct_src = np.asarray(inp['ct_src']); ct_dst = np.asarray(inp['ct_dst'])

    def _remap512(g, local):
        # table rows after 512-row piecewise AllGathers: piece p holds local
        # rows p*512:(p+1)*512 of every rank at tab[p*4096 + r*512 + (l%512)].
        r = g // local
        l = g % local
        return (l // 512) * (NCORES * 512) + r * 512 + (l % 512)
    cc_src = _remap512(cc_src, NCL)
    ct_src = _remap512(ct_src, NCL)
    starts_t = _remap512(starts, NTL)
    ends_t = _remap512(ends, NTL)
    C_CC, cc_meta = _prep_edges([cc_src[s] for s in range(4)], [cc_dst[s] for s in range(4)], WCC)
    C_CT, ct_meta = _prep_edges([ct_src], [ct_dst], WCT)

    ident = np.eye(128, dtype=F32).astype(BF)
    ones1 = np.ones((1, 128), F32).astype(BF)
    iota = np.tile(np.arange(128, dtype=F32), (128, 1)).astype(BF)

    in_maps = []
    for c in range(NCORES):
        csl = slice(c * NCL, (c + 1) * NCL)
        tsl = slice(c * NTL, (c + 1) * NTL)
        m = dict(
            consT0=np.ascontiguousarray(cons[csl].T).astype(F8),       # [1664, 2048]
            labelT=np.ascontiguousarray(label_emb[csl].T).astype(F8),  # [128, 2048]
            tokT=np.ascontiguousarray(tok[tsl].T).astype(BF),          # [768, 1024]
            tok_f32=np.ascontiguousarray(tok[tsl]),                    # [1024, 768]
            idx_starts=_idx_plane(starts_t[csl].astype(np.int16)),     # [128, 128]
            idx_ends=_idx_plane(ends_t[csl].astype(np.int16)),
            wf_ext=Wf_ext, wb_ext=Wb_ext, wk_ext=Wk_ext, wq_ad=Wq_ad,
            w1=w1, w2r=w2r, b1c=b1c, fuse_w=fuse_w, fb_row=fb_row,
            ident=ident, ones1=ones1, iota=iota,
            idx_cc=cc_meta[c]['idx'], dl_cc=cc_meta[c]['dl'], ohd_cc=cc_meta[c]['ohd'],
            idx_ct=ct_meta[c]['idx'], dl_ct=ct_meta[c]['dl'], ohd_ct=ct_meta[c]['ohd'],
        )
        in_maps.append(m)
    return in_maps, C_CC, C_CT


# ------------------------------------------------------------- device build --

def _build_nc(C_CC, C_CT):
    ECC = C_CC * 128
    ECT = C_CT * 128
    nc = bacc.Bacc("TRN2", num_devices=NCORES)

    def ein(name, shape, dt_):
        return nc.dram_tensor(name, shape, dt_, kind="ExternalInput")

    consT0 = ein("consT0", [D, NCL], fp8)
    labelT = ein("labelT", [128, NCL], fp8)
    tokT = ein("tokT", [DH, NTL], bf16)
    tok_f32_d = ein("tok_f32", [NTL, DH], f32)
    idx_starts = ein("idx_starts", [128, NCL // 16], i16)
    idx_ends = ein("idx_ends", [128, NCL // 16], i16)
    wf_ext = ein("wf_ext", [D, 1680], fp8)
    wb_ext = ein("wb_ext", [D, 1680], fp8)
    wk_ext = ein("wk_ext", [D, 784], fp8)
    wq_ad = ein("wq_ad", [DH, 8], bf16)
    w1_d = ein("w1", [D, 1024], fp8)
    w2r_d = ein("w2r", [128, 2048], bf16)
    b1c_d = ein("b1c", [128, 8], f32)
    fuse_w_d = ein("fuse_w", [2 * DH, DH], bf16)
    fb_row_d = ein("fb_row", [1, DH], bf16)
    ident_d = ein("ident", [128, 128], bf16)
    ones1_d = ein("ones1", [1, 128], bf16)
    iota_d = ein("iota", [128, 128], bf16)
    idx_cc_d = ein("idx_cc", [128, 4 * WCC * (ECC // 16)], i16)
    dl_cc_d = ein("dl_cc", [128, 4 * WCC * C_CC], bf16)
    ohd_cc_d = ein("ohd_cc", [128, 4 * WCC * ECC], bf16)
    idx_ct_d = ein("idx_ct", [128, WCT * (ECT // 16)], i16)
    dl_ct_d = ein("dl_ct", [128, WCT * C_CT], bf16)
    ohd_ct_d = ein("ohd_ct", [128, WCT * ECT], bf16)

    out_d = nc.dram_tensor("out", [NTL, DH], f32, kind="ExternalOutput")
    DBG = os.environ.get("DEBUG_TAPS", "0") == "1"
    if DBG:
        dbg_o0 = nc.dram_tensor("dbg_o0", [NCL, D], bf16, kind="ExternalOutput")
        dbg_o1 = nc.dram_tensor("dbg_o1", [NCL, D], bf16, kind="ExternalOutput")
        dbg_tokc = nc.dram_tensor("dbg_tokc", [NTL, DH], f32, kind="ExternalOutput")

    tf_in = nc.dram_tensor("tf_in", [NCL, SWF], fp8)
    tb_in = nc.dram_tensor("tb_in", [NCL, SWF], fp8)
    tk_in = nc.dram_tensor("tk_in", [NCL, SWK], fp8)
    tt_in = nc.dram_tensor("tt_in", [NTL, DH], bf16)
    tf_tab = nc.dram_tensor("tf_tab", [NC, SWF], fp8, addr_space="Shared")
    tb_tab = nc.dram_tensor("tb_tab", [NC, SWF], fp8, addr_space="Shared")
    tk_tab = nc.dram_tensor("tk_tab", [NC, SWK], fp8, addr_space="Shared")
    tt_tab = nc.dram_tensor("tt_tab", [NT, DH], bf16, addr_space="Shared")
    ord_dram = [[nc.dram_tensor(f"ordp{j}_{q}", [512, D], bf16) for q in range(4)]
                for j in range(2)]
    tkc_dram = nc.dram_tensor("tkc", [NTL, DH], bf16)
    tcf_dram = nc.dram_tensor("tcf", [NTL, DH], f32)

    RG = [list(range(NCORES))]

    with tile.TileContext(nc) as tc:
      with tc.tile_pool(name="const", bufs=1) as cp, \
           tc.tile_pool(name="keep1", bufs=1) as kp1, \
           tc.tile_pool(name="keep2", bufs=2) as kp2:
        def cload(name, dram, shape, dt_):
            t = cp.tile(shape, dt_, name=name)
            nc.sync.dma_start(out=t[:], in_=dram[:])
            return t
        ident_t = cload("ident", ident_d, [128, 128], bf16)
        ones1_t = cload("ones1", ones1_d, [1, 128], bf16)
        iota_t = cload("iota", iota_d, [128, 128], bf16)
        idx_cc_t = cload("idx_cc", idx_cc_d, [128, 4 * WCC * (ECC // 16)], i16)
        dl_cc_t = cload("dl_cc", dl_cc_d, [128, 4 * WCC * C_CC], bf16)
        idx_ct_t = cload("idx_ct", idx_ct_d, [128, WCT * (ECT // 16)], i16)
        dl_ct_t = cload("dl_ct", dl_ct_d, [128, WCT * C_CT], bf16)
        w2r_t = cload("w2r", w2r_d, [128, 2048], bf16)
        b1c_t = cload("b1c", b1c_d, [128, 8], f32)

        DR = mybir.MatmulPerfMode.DoubleRow

        def gemm_block(ps, consT, m, w_t, nchunks, outs, tag):
            # fp8 DoubleRow over k-tile pairs (2x ALU), k-outer / n-inner over
            # one 4-bank PSUM tile (one open accumulation group per bank).
            pt = ps.tile([128, 2048], f32, name=f"gp{tag}")
            msl = slice(m * 128, (m + 1) * 128)
            for kp in range(6):
                for (n0, nw) in nchunks:
                    nc.tensor.matmul(out=pt[:, n0:n0 + nw],
                                     lhsT=consT[:, 2 * kp:2 * kp + 2, msl],
                                     rhs=w_t[:, 2 * kp:2 * kp + 2, n0:n0 + nw],
                                     perf_mode=DR,
                                     start=(kp == 0), stop=False,
                                     skip_group_check=True)
            for (n0, nw) in nchunks:
                nc.tensor.matmul(out=pt[:, n0:n0 + nw], lhsT=consT[:, 12, msl],
                                 rhs=w_t[:, 12, n0:n0 + nw],
                                 start=False, stop=True, skip_group_check=True)
            for (n0, nw, out_ap) in outs:
                nc.scalar.activation(out=out_ap, in_=pt[:, n0:n0 + nw], func=AF.Copy)

        def gat_pass(sb, ps_pv, ps_sm, tab, stride, sdoff, idx_t, dl_t, ohd_d,
                     sd_tile, wins, C, dfeat, blk0, writeback,
                     wb_final=lambda w, sb: None, tag=""):
            E = C * 128
            dh = dfeat // 8
            gstep = 5
            for w in range(wins):
                blk = blk0 + w
                segs = []
                for c0 in range(0, C, gstep):
                    cc = min(gstep, C - c0)
                    ioff = blk * (E // 16) + c0 * 8
                    gt = sb.tile([128, cc, stride], fp8, name=f"gw{tag}{c0}")
                    nc.gpsimd.dma_gather(
                        out_ap=gt[:, 0:cc, :], in_ap=tab[:],
                        idxs_ap=idx_t[:, ioff:ioff + cc * 8],
                        num_idxs=cc * 128, num_idxs_reg=cc * 128, elem_size=stride)
                    segs.append((c0, cc, gt))

                def gch(c):
                    for (c0, cc, gt) in segs:
                        if c0 <= c < c0 + cc:
                            return gt[:, c - c0, :]
                # edge-side one-hot generated on-chip; dst-side from host
                ohw = sb.tile([128, C, 128], bf16, name=f"oh{tag}")
                nc.vector.tensor_tensor(
                    out=ohw[:],
                    in0=dl_t[:, blk * C:(blk + 1) * C]
                        .unsqueeze(2).broadcast_to([128, C, 128]),
                    in1=iota_t[:].unsqueeze(1).broadcast_to([128, C, 128]),
                    op=OP.is_equal)
                ohdw = sb.tile([128, C, 128], bf16, name=f"ohd{tag}")
                nc.sync.dma_start(out=ohdw[:],
                                  in_=ohd_d[:, blk * E:(blk + 1) * E]
                                      .rearrange("p (c e) -> p c e", c=C))
                # logit strip: one-hot scatter of dst-side sd + gathered src sd
                strip = ps_sm.tile([128, 512], f32, name=f"strip{tag}")
                for c in range(C):
                    nc.tensor.matmul(out=strip[:, c * 8:(c + 1) * 8], lhsT=ohdw[:, c, :],
                                     rhs=sd_tile[:, w, :], start=True, stop=True)
                lgs = sb.tile([128, C, 8], f32, name=f"lgs{tag}")
                for (c0, cc, gt) in segs:
                    gtb = gt[:].bitcast(bf16)          # [128, cc, stride//2]
                    nc.vector.tensor_tensor(
                        out=lgs[:, c0:c0 + cc, :],
                        in0=strip[:, c0 * 8:(c0 + cc) * 8]
                            .rearrange("p (c e) -> p c e", c=cc),
                        in1=gtb[:, 0:cc, sdoff:sdoff + 8],
                        op=OP.add)
                lg = sb.tile([128, C * 8], f32, name=f"lg{tag}")
                nc.scalar.activation(out=lg[:], in_=lgs[:].rearrange("p c e -> p (c e)"),
                                     func=AF.Prelu, alpha=0.2)
                ex = sb.tile([128, C, 8], bf16, name=f"ex{tag}")
                nc.scalar.activation(out=ex[:].rearrange("p c e -> p (c e)"), in_=lg[:],
                                     func=AF.Exp)
                # alpha folded into the one-hot planes (per head), values
                # stream into PE raw (fp8 rhs x bf16 lhsT).
                oha = sb.tile([128, C, 8, 128], bf16, name=f"oha{tag}")
                nc.vector.tensor_tensor(
                    out=oha[:],
                    in0=ohw[:].unsqueeze(2).broadcast_to([128, C, 8, 128]),
                    in1=ex[:].unsqueeze(3).broadcast_to([128, C, 8, 128]),
                    op=OP.mult)
                rec = None
                for hp in range(4):
                    # one open accumulation group per PSUM bank (512 f32/head)
                    pv = ps_pv.tile([128, 2, 512], f32, name=f"pv{tag}")
                    for c in range(C):
                        for hi in range(2):
                            h = hp * 2 + hi
                            nc.tensor.matmul(out=pv[:, hi, 0:dh], lhsT=oha[:, c, h, :],
                                             rhs=gch(c)[:, h * dh:(h + 1) * dh],
                                             start=(c == 0), stop=(c == C - 1),
                                             skip_group_check=True)
                        if hp == 0:
                            nc.tensor.matmul(out=strip[:, 384:392], lhsT=ohw[:, c, :],
                                             rhs=ex[:, c, :], start=(c == 0), stop=(c == C - 1))
                    if hp == 0:
                        dent = sb.tile([128, 8], f32, name=f"dent{tag}")
                        nc.vector.tensor_scalar(out=dent[:], in0=strip[:, 384:392],
                                                scalar1=1e-9, scalar2=None, op0=OP.add)
                        rec = sb.tile([128, 8], f32, name=f"rec{tag}")
                        nc.vector.reciprocal(out=rec[:], in_=dent[:])
                    writeback(w, hp, pv, rec, sb)
                wb_final(w, sb)

        # ============================ iterations ============================
        PH = int(os.environ.get("PHASE_LIMIT", "99"))
        NITER = int(os.environ.get("NITER", "2"))
        for it in range(NITER):
            # -------- P1+P2: consT, hf/hb tables, AllGather -----------------
            with tc.tile_pool(name=f"p2_{it}", bufs=1) as sb2, \
                 tc.tile_pool(name=f"p2s_{it}", bufs=3) as sb2s, \
                 tc.tile_pool(name=f"p2p_{it}", bufs=2, space="PSUM") as ps2:
                consT = sb2.tile([128, 13, NCL], fp8, name="consT")
                if it == 0:
                    nc.sync.dma_start(out=consT[:],
                                      in_=consT0[:].rearrange("(k p) e -> p k e", p=128))
                else:
                    ist = sb2.tile([128, NCL // 16], i16, name="ist")
                    nc.sync.dma_start(out=ist[:], in_=idx_starts[:])
                    ien = sb2.tile([128, NCL // 16], i16, name="ien")
                    nc.sync.dma_start(out=ien[:], in_=idx_ends[:])
                    for half, idxt in ((0, ist), (1, ien)):
                        for q in range(4):
                            gt = sb2s.tile([128, 6, 512], bf16, name="gtc")
                            nc.gpsimd.dma_gather(
                                out_ap=gt[:], in_ap=tt_tab[:],
                                idxs_ap=idxt[:, q * 32:(q + 1) * 32],
                                num_idxs=512, num_idxs_reg=512,
                                elem_size=DH, transpose=True)
                            nc.vector.tensor_copy(
                                out=consT[:, 6 * half:6 * half + 6, q * 512:(q + 1) * 512],
                                in_=gt[:])
                    nc.sync.dma_start(out=consT[:, 12, :], in_=labelT[:])
                sd_f = kp1.tile([128, WCC, 8], bf16, name=f"sd_f{it}")
                sd_b = kp1.tile([128, WCC, 8], bf16, name=f"sd_b{it}")
                for (wd, outd, tabd, sdt) in [(wf_ext, tf_in, tf_tab, sd_f),
                                              (wb_ext, tb_in, tb_tab, sd_b)]:
                    w_t = sb2.tile([128, 13, 1680], fp8, name="wtab")
                    nc.sync.dma_start(out=w_t[:], in_=wd[:].rearrange("(k p) n -> p k n", p=128))
                    for m in range(WCC):
                        stgv = sb2s.tile([128, 1664], fp8, name="stgv")
                        stgs = sb2s.tile([128, 16], bf16, name="stgs")
                        gemm_block(ps2, consT, m,
                                   w_t, [(0, 512), (512, 512), (1024, 512), (1536, 144)],
                                   [(0, 512, stgv[:, 0:512]),
                                    (512, 512, stgv[:, 512:1024]),
                                    (1024, 512, stgv[:, 1024:1536]),
                                    (1536, 128, stgv[:, 1536:1664]),
                                    (1664, 16, stgs[:])], "t")
                        nc.sync.dma_start(out=outd[m * 128:(m + 1) * 128, 0:1664],
                                          in_=stgv[:])
                        nc.sync.dma_start(
                            out=outd[m * 128:(m + 1) * 128, 1664:1696].bitcast(bf16),
                            in_=stgs[:])
                        nc.vector.tensor_copy(out=sdt[:, m, :], in_=stgs[:, 8:16])
                        if m % 4 == 3:
                            q = m // 4
                            nc.gpsimd.collective_compute(
                                "AllGather", OP.bypass, replica_groups=RG,
                                ins=[outd[q * 512:(q + 1) * 512, :]],
                                outs=[tabd[q * 4096:(q + 1) * 4096, :]])

            # -------- P5b: sd_q = tok_cons @ Wq_ad --------------------------
            with tc.tile_pool(name=f"p5b_{it}", bufs=1) as sb5b, \
                 tc.tile_pool(name=f"p5bp_{it}", bufs=2, space="PSUM") as ps5b:
                tokcT = kp2.tile([128, 6, NTL], bf16, name="tokcT")
                if it == 0:
                    nc.sync.dma_start(out=tokcT[:],
                                      in_=tokT[:].rearrange("(k p) e -> p k e", p=128))
                else:
                    nc.sync.dma_start_transpose(out=tokcT[:], in_=tkc_dram[:])
                wq_t = sb5b.tile([128, 6, 8], bf16, name="wqt")
                nc.sync.dma_start(out=wq_t[:], in_=wq_ad[:].rearrange("(k p) n -> p k n", p=128))
                sd_q = kp1.tile([128, WCT, 8], bf16, name=f"sd_q{it}")
                for m in range(WCT):
                    pq = ps5b.tile([128, 16], f32, name="pq")
                    for k in range(6):
                        nc.tensor.matmul(out=pq[:, :8], lhsT=tokcT[:, k, m * 128:(m + 1) * 128],
                                         rhs=wq_t[:, k, :], start=(k == 0), stop=(k == 5))
                    nc.scalar.activation(out=sd_q[:, m, :], in_=pq[:, :8], func=AF.Copy)

            # -------- P3: cc GATs -> order_j, plain spill to ord_dram -------
            if PH < 3: break
            with tc.tile_pool(name=f"p3_{it}", bufs=1) as sb3o, \
                 tc.tile_pool(name=f"p3s_{it}", bufs=4) as sb3, \
                 tc.tile_pool(name=f"p3pv_{it}", bufs=3, space="PSUM") as ps3a, \
                 tc.tile_pool(name=f"p3ps_{it}", bufs=2, space="PSUM") as ps3b:

                def mk_wb_first(ordt):
                    def wb_first(w, hp, pv, rec, sb):
                        nc.vector.tensor_tensor(
                            out=ordt[:, w, hp * 416:(hp + 1) * 416]
                                .rearrange("p (h d) -> p h d", h=2),
                            in0=pv[:, :, 0:208],
                            in1=rec[:, hp * 2:(hp + 1) * 2]
                                .unsqueeze(2).broadcast_to([128, 2, 208]),
                            op=OP.mult)
                    return wb_first

                def mk_wb_add(ordt):
                    def wb_add(w, hp, pv, rec, sb):
                        t = sb.tile([128, 416], bf16, name="tadd")
                        nc.vector.tensor_tensor(
                            out=t[:].rearrange("p (h d) -> p h d", h=2),
                            in0=pv[:, :, 0:208],
                            in1=rec[:, hp * 2:(hp + 1) * 2]
                                .unsqueeze(2).broadcast_to([128, 2, 208]),
                            op=OP.mult)
                        nc.vector.tensor_tensor(
                            out=ordt[:, w, hp * 416:(hp + 1) * 416],
                            in0=ordt[:, w, hp * 416:(hp + 1) * 416],
                            in1=t[:], op=OP.add)
                    return wb_add

                def mk_wb_fin(ordt, j):
                    def wb_fin(w, sb):
                        nc.sync.dma_start(
                            out=ord_dram[j][w // 4][(w % 4) * 128:(w % 4 + 1) * 128, :],
                            in_=ordt[:, w, :])
                    return wb_fin

                ordt0 = sb3o.tile([128, WCC, D], bf16, name="ordt")
                gat_pass(sb3, ps3a, ps3b, tf_tab, SWF, SDF, idx_cc_t, dl_cc_t, ohd_cc_d,
                         sd_f, WCC, C_CC, D, blk0=0,
                         writeback=mk_wb_first(ordt0), tag="cc")
                for q in range(4):
                    nc.sync.dma_start(
                        out=ord_dram[0][q][:].rearrange("(w p) d -> p w d", p=128),
                        in_=ordt0[:, q * 4:(q + 1) * 4, :])
                ordt1 = sb3o.tile([128, WCC, D], bf16, name="ordt")
                gat_pass(sb3, ps3a, ps3b, tf_tab, SWF, SDF, idx_cc_t, dl_cc_t, ohd_cc_d,
                         sd_f, WCC, C_CC, D, blk0=WCC,
                         writeback=mk_wb_first(ordt1), tag="cc")
                gat_pass(sb3, ps3a, ps3b, tb_tab, SWF, SDF, idx_cc_t, dl_cc_t, ohd_cc_d,
                         sd_b, WCC, C_CC, D, blk0=3 * WCC,
                         writeback=mk_wb_add(ordt1), wb_final=mk_wb_fin(ordt1, 1),
                         tag="cc")
                ordt0b = sb3o.tile([128, WCC, D], bf16, name="ordt")
                for q in range(4):
                    nc.sync.dma_start(
                        out=ordt0b[:, q * 4:(q + 1) * 4, :],
                        in_=ord_dram[0][q][:].rearrange("(w p) d -> p w d", p=128))
                gat_pass(sb3, ps3a, ps3b, tb_tab, SWF, SDF, idx_cc_t, dl_cc_t, ohd_cc_d,
                         sd_b, WCC, C_CC, D, blk0=2 * WCC,
                         writeback=mk_wb_add(ordt0b), wb_final=mk_wb_fin(ordt0b, 0),
                         tag="cc")

            # -------- P4: attention scores + gate + single Wk GEMM ----------
            if PH < 4: break
            with tc.tile_pool(name=f"p4_{it}", bufs=1) as sb4, \
                 tc.tile_pool(name=f"p4s_{it}", bufs=2) as sb4s, \
                 tc.tile_pool(name=f"p4p_{it}", bufs=2, space="PSUM") as ps4, \
                 tc.tile_pool(name=f"p4pr_{it}", bufs=1, space="PSUM") as ps4r:
                w1_t = sb4.tile([128, 13, 1024], fp8, name="w1t")
                nc.sync.dma_start(out=w1_t[:], in_=w1_d[:].rearrange("(k p) n -> p k n", p=128))
                wk_t = sb4.tile([128, 13, 784], fp8, name="wkt")
                nc.sync.dma_start(out=wk_t[:], in_=wk_ext[:].rearrange("(k p) n -> p k n", p=128))
                for q in range(4):
                    sTq = []
                    s8q = []
                    h1q = []
                    for j in range(2):
                        sT = sb4s.tile([128, 13, 512], bf16, name=f"sT{j}")
                        nc.sync.dma_start_transpose(
                            out=sT[:], in_=ord_dram[j][q][:])
                        sTq.append(sT)
                        s8 = sb4s.tile([128, 13, 512], fp8, name=f"s8{j}")
                        nc.vector.tensor_copy(out=s8[:], in_=sT[:])
                        s8q.append(s8)
                        h1 = sb4s.tile([128, 8, 512], bf16, name=f"h1_{j}")
                        for t in range(8):
                            ph = ps4.tile([128, 512], f32, name="ph")
                            for kp in range(6):
                                nc.tensor.matmul(
                                    out=ph[:],
                                    lhsT=w1_t[:, 2 * kp:2 * kp + 2, t * 128:(t + 1) * 128],
                                    rhs=s8[:, 2 * kp:2 * kp + 2, :],
                                    perf_mode=DR, start=(kp == 0), stop=False,
                                    skip_group_check=True)
                            nc.tensor.matmul(out=ph[:], lhsT=w1_t[:, 12, t * 128:(t + 1) * 128],
                                             rhs=s8[:, 12, :], start=False, stop=True,
                                             skip_group_check=True)
                            nc.scalar.activation(out=h1[:, t, :], in_=ph[:], func=AF.Relu,
                                                 bias=b1c_t[:, t:t + 1])
                        h1q.append(h1)
                    psc = ps4r.tile([128, 512], f32, name="psc")
                    for j in range(2):
                        for t in range(8):
                            nc.tensor.matmul(out=psc[:],
                                             lhsT=w2r_t[:, j * 1024 + t * 128:
                                                        j * 1024 + (t + 1) * 128],
                                             rhs=h1q[j][:, t, :],
                                             start=(j == 0 and t == 0),
                                             stop=(j == 1 and t == 7))
                    w0b = sb4s.tile([128, 512], bf16, name="w0b")
                    nc.scalar.activation(out=w0b[:], in_=psc[:], func=AF.Sigmoid)
                    # gate combine: sTc8 <- fp8(sT1 + w0*(sT0-sT1))
                    sTc = sTq[0]
                    nc.vector.tensor_tensor(out=sTc[:], in0=sTc[:], in1=sTq[1][:],
                                            op=OP.subtract)
                    nc.vector.tensor_tensor(
                        out=sTc[:], in0=sTc[:],
                        in1=w0b[:].unsqueeze(1).broadcast_to([128, 13, 512]),
                        op=OP.mult)
                    sTc8 = s8q[0]
                    nc.vector.tensor_tensor(out=sTc8[:], in0=sTc[:], in1=sTq[1][:], op=OP.add)
                    for b in range(4):
                        m = q * 4 + b
                        stghv = sb4s.tile([128, 768], fp8, name="stghv")
                        stghs = sb4s.tile([128, 8], bf16, name="stghs")
                        pk = ps4.tile([128, 1024], f32, name="pk")
                        for kp in range(6):
                            for (n0, nw) in [(0, 512), (512, 272)]:
                                nc.tensor.matmul(
                                    out=pk[:, n0:n0 + nw],
                                    lhsT=sTc8[:, 2 * kp:2 * kp + 2, b * 128:(b + 1) * 128],
                                    rhs=wk_t[:, 2 * kp:2 * kp + 2, n0:n0 + nw],
                                    perf_mode=DR, start=(kp == 0), stop=False,
                                    skip_group_check=True)
                        for (n0, nw) in [(0, 512), (512, 272)]:
                            nc.tensor.matmul(out=pk[:, n0:n0 + nw],
                                             lhsT=sTc8[:, 12, b * 128:(b + 1) * 128],
                                             rhs=wk_t[:, 12, n0:n0 + nw],
                                             start=False, stop=True,
                                             skip_group_check=True)
                        nc.scalar.activation(out=stghv[:, 0:512], in_=pk[:, 0:512],
                                             func=AF.Copy)
                        nc.scalar.activation(out=stghv[:, 512:768], in_=pk[:, 512:768],
                                             func=AF.Copy)
                        nc.scalar.activation(out=stghs[:], in_=pk[:, 768:776],
                                             func=AF.Copy)
                        nc.sync.dma_start(out=tk_in[m * 128:(m + 1) * 128, 0:768],
                                          in_=stghv[:])
                        nc.sync.dma_start(
                            out=tk_in[m * 128:(m + 1) * 128, 768:784].bitcast(bf16),
                            in_=stghs[:])
                    nc.gpsimd.collective_compute(
                        "AllGather", OP.bypass, replica_groups=RG,
                        ins=[tk_in[q * 512:(q + 1) * 512, :]],
                        outs=[tk_tab[q * 4096:(q + 1) * 4096, :]])

            # -------- P6: ct GAT -> tok_cons --------------------------------
            if PH < 6: break
            with tc.tile_pool(name=f"p6_{it}", bufs=2) as sb6, \
                 tc.tile_pool(name=f"p6pv_{it}", bufs=3, space="PSUM") as ps6a, \
                 tc.tile_pool(name=f"p6ps_{it}", bufs=2, space="PSUM") as ps6b:

                tkw_tiles = {}

                def wb_tok(w, hp, pv, rec, sb):
                    if hp == 0:
                        tkw_tiles[w] = sb.tile([128, DH], f32, name="tkw")
                    tkw = tkw_tiles[w]
                    nc.vector.tensor_tensor(
                        out=tkw[:, hp * 192:(hp + 1) * 192].rearrange("p (h d) -> p h d", h=2),
                        in0=pv[:, :, 0:96],
                        in1=rec[:, hp * 2:(hp + 1) * 2]
                            .unsqueeze(2).broadcast_to([128, 2, 96]),
                        op=OP.mult)

                def wb_tok_fin(w, sb):
                    tkw = tkw_tiles.pop(w)
                    if it == NITER - 1:
                        nc.sync.dma_start(out=tcf_dram[w * 128:(w + 1) * 128, :], in_=tkw[:])
                    stg = sb.tile([128, DH], bf16, name="stgt")
                    nc.vector.tensor_copy(out=stg[:], in_=tkw[:])
                    nc.sync.dma_start(out=tkc_dram[w * 128:(w + 1) * 128, :], in_=stg[:])
                    if it == 0:
                        nc.sync.dma_start(out=tt_in[w * 128:(w + 1) * 128, :], in_=stg[:])

                def wb_tok_fin2(w, sb):
                    wb_tok_fin(w, sb)
                    if it == 0 and NITER > 1 and w in (3, 7):
                        hof = (w // 4) * 512
                        nc.gpsimd.collective_compute(
                            "AllGather", OP.bypass, replica_groups=RG,
                            ins=[tt_in[hof:hof + 512, :]],
                            outs=[tt_tab[hof * NCORES:(hof + 512) * NCORES, :]])

                gat_pass(sb6, ps6a, ps6b, tk_tab, SWK, SDK, idx_ct_t, dl_ct_t, ohd_ct_d,
                         sd_q, WCT, C_CT, DH, blk0=0, writeback=wb_tok,
                         wb_final=wb_tok_fin2, tag="ct")

        if DBG:
            for q in range(4):
                nc.sync.dma_start(out=dbg_o0[q * 512:(q + 1) * 512, :],
                                  in_=ord_dram[0][q][:])
                nc.sync.dma_start(out=dbg_o1[q * 512:(q + 1) * 512, :],
                                  in_=ord_dram[1][q][:])
            nc.sync.dma_start(out=dbg_tokc[:], in_=tcf_dram[:])

        # -------- P7: fuse gate + blend -------------------------------------
        if PH >= 7:
          with tc.tile_pool(name="p7", bufs=1) as sb7, \
              tc.tile_pool(name="p7s", bufs=3) as sb7s, \
              tc.tile_pool(name="p7p", bufs=2, space="PSUM") as ps7:
             fw_t = sb7.tile([128, 12, DH], bf16, name="fwt")
             nc.sync.dma_start(out=fw_t[:], in_=fuse_w_d[:].rearrange("(k p) n -> p k n", p=128))
             fb_t = sb7.tile([1, DH], bf16, name="fbt")
             nc.sync.dma_start(out=fb_t[:], in_=fb_row_d[:])
             tokT_t = sb7.tile([128, 6, NTL], bf16, name="tokTt")
             nc.sync.dma_start(out=tokT_t[:], in_=tokT[:].rearrange("(k p) e -> p k e", p=128))
             for m in range(WCT):
                 tokcT_m = sb7s.tile([128, 6, 128], bf16, name="tokcTm")
                 nc.sync.dma_start_transpose(out=tokcT_m[:],
                                             in_=tkc_dram[m * 128:(m + 1) * 128, :])
                 f_t = sb7s.tile([128, DH], f32, name="f_t")
                 for (n0, nw) in [(0, 512), (512, 256)]:
                     pf = ps7.tile([128, 512], f32, name="pf")
                     for k in range(12):
                         lt = tokT_t[:, k, m * 128:(m + 1) * 128] if k < 6 else \
                              tokcT_m[:, k - 6, :]
                         nc.tensor.matmul(out=pf[:, :nw], lhsT=lt, rhs=fw_t[:, k, n0:n0 + nw],
                                          start=(k == 0), stop=False)
                     nc.tensor.matmul(out=pf[:, :nw], lhsT=ones1_t[:], rhs=fb_t[:, n0:n0 + nw],
                                      start=False, stop=True)
                     nc.scalar.activation(out=f_t[:, n0:n0 + nw], in_=pf[:, :nw], func=AF.Sigmoid)
                 tok_in = sb7s.tile([128, DH], f32, name="tok_in")
                 nc.sync.dma_start(out=tok_in[:], in_=tok_f32_d[m * 128:(m + 1) * 128, :])
                 tcf_in = sb7s.tile([128, DH], f32, name="tcf_in")
                 nc.sync.dma_start(out=tcf_in[:], in_=tcf_dram[m * 128:(m + 1) * 128, :])
                 dlt = sb7s.tile([128, DH], f32, name="dlt")
                 nc.vector.tensor_tensor(out=dlt[:], in0=tok_in[:], in1=tcf_in[:], op=OP.subtract)
                 nc.vector.tensor_tensor(out=dlt[:], in0=f_t[:], in1=dlt[:], op=OP.mult)
                 nc.vector.tensor_tensor(out=dlt[:], in0=tcf_in[:], in1=dlt[:], op=OP.add)
                 nc.sync.dma_start(out=out_d[m * 128:(m + 1) * 128, :], in_=dlt[:])

    nc.finalize()
    return nc


# ------------------------------------------------------------------ driver --

def run(inputs, trace=False):
    in_maps, C_CC, C_CT = _host_prep(inputs)
    nc = _build_nc(C_CC, C_CT)
    res = run_bass_kernel_spmd(nc, in_maps, core_ids=list(range(NCORES)), trace=trace)
    out = np.concatenate([res.results[c]["out"] for c in range(NCORES)], axis=0)
    if os.environ.get("DEBUG_TAPS", "0") == "1":
        run.dbg = {k: np.concatenate([res.results[c][k] for c in range(NCORES)], axis=0)
                   for k in ("dbg_o0", "dbg_o1", "dbg_tokc")}
    return out.astype(np.float32), res


def kernel(**inputs) -> np.ndarray:
    out, _ = run(inputs, trace=False)
    return out
